# revision 20
# baseline (speedup 1.0000x reference)
"""Trainium2 Bass kernel for nn_Model_39676907886571 (per-head attention, S=2048, d=3).

Math (per head h, fully head/data parallel, one head per NeuronCore):
  q_mat = query[h] @ x[h].T          (3, S)   -> q = q_mat viewed row-major as (S, 3)
  k_mat, v_mat likewise (the reshape is a memory-reinterpreting view, not a transpose)
  attn  = softmax(q @ k.T / sqrt(3)) (S, S)
  out   = (attn @ v).T               (3, S)

Device strategy (all on-chip, the S x S attention matrix never touches HBM):
  * The memory-reinterpreting (S,3)-view of the (3,S) projection output is a
    fixed permutation of x entries, so it is absorbed into the HOST-side input
    layout: a zero-masked 27-partition arrangement of x (k = (p', i, d) with
    the view's row-crossing i-mask baked in) turns q^T and k^T into plain
    [27,3] x [27,512] PE matmuls whose outputs are ALREADY (3, S)-transposed
    and in true sequence order.  The [1|v0|v1|v2] quads for attn@[1|v]
    likewise come from two 82-dim masked matmuls.  This removes the
    DRAM-bounce reshape, all 40 PE transposes, and the u-order un-permute of
    the earlier design; both softmax axes are true-order so the output DMA
    streams per s-chunk.
  * E^T = exp(k-chunks^T @ q^T / sqrt(3)) keeps the key axis on partitions, so
    attn @ [1|v] needs no transposes and the softmax denominator falls out of
    the ones column of the [1|v] stationary operand.
  * Matmul operands are float32r (single-pass fp32 PE mode); PSUM: 2 x 3-bank
    ping-pong qk tiles + 2 x 1-bank attn@[1|v] accumulators; the acc banks
    double as staging for the later projection pieces before their first
    accumulation starts (matmul outputs must sit at base partition 0).
  * Software pipelining: mm1 of round g+1 issues before mm2 of round g so the
    in-order PE stream overlaps the exp; ACT (exp of the 2048^2 attention
    matrix = ~31.7us floor) runs back-to-back from round 0, which starts as
    soon as the first 512-col projection pieces are copied to SBUF.  Round
    widths are 3,3,2,3,3,2 chunks (a width-1 round mid-stream starves the
    next round's mm1 behind the 2-tile PSUM ping-pong); only the program's
    last round is width 1, keeping the tail (mm2 + normalize + final DMA)
    short.  The tail normalization runs ACT (acc->SBUF copy) and DVE
    (reciprocal) in parallel, broadcasts via a PE matmul, and multiplies on
    DVE with a single PSUM operand.
"""

import numpy as np
from contextlib import ExitStack

import concourse.bass as bass
import concourse.tile as tile
from concourse import bacc, mybir
from concourse import bass_utils

F32 = mybir.dt.float32
F32R = mybir.dt.float32r

H, S, D = 8, 2048, 3
NCH = 16                # t-chunks of 128
SQ = 512                # s-chunk width (one PSUM bank)
INV_SCALE = float(1.0 / np.sqrt(3.0))

RC_EARLY = [(0, 1), (2, 3, 4), (5, 6, 7), (8, 9, 10), (11, 12, 13), (14, 15)]
RC_LAST = [(0, 1, 2), (3, 4, 5), (6, 7, 8), (9, 10, 11), (12, 13, 14), (15,)]
NR = 6
NG = 4 * NR


def _rc(j):
    return RC_LAST if j == 3 else RC_EARLY


def _r(ap):
    """Bitcast an fp32 AP to float32r (same bits)."""
    return ap.bitcast(F32R)


def build_program(reps=1):
    nc = bacc.Bacc("TRN2", num_devices=H, debug=False)
    xw_dram = nc.dram_tensor("xw", (27, 2054), F32, kind="ExternalInput")
    xv_dram = nc.dram_tensor("xv", (82, 320), F32, kind="ExternalInput")
    out_dram = nc.dram_tensor("out", (3, S), F32, kind="ExternalOutput")

    with tile.TileContext(nc) as tc, ExitStack() as ctx:
        consts = ctx.enter_context(tc.tile_pool(name="consts", bufs=1))
        sb = ctx.enter_context(tc.tile_pool(name="sb", bufs=2 if reps > 1 else 1))
        es = ctx.enter_context(tc.tile_pool(name="es", bufs=4))
        ping = ctx.enter_context(tc.tile_pool(name="ping", bufs=1, space="PSUM"))
        accp = ctx.enter_context(tc.tile_pool(name="accp", bufs=1, space="PSUM"))

        # constants (shared across reps)
        warm = consts.tile([1, 4], F32)
        nc.vector.memset(warm, 1.0)
        ones4 = consts.tile([1, 4], F32R)
        nc.vector.tensor_copy(ones4[:], warm[:])
        # prewarm the ACT exp table so the ~1.3us table load overlaps the prologue
        nc.scalar.activation(warm[0:1, 0:1], warm[0:1, 1:2],
                             mybir.ActivationFunctionType.Exp)
        # memset on the otherwise-idle GPSIMD so the dead warm-up transposes
        # are ready almost at t=0 (the PE pstate ramp clock starts then)
        wtile = consts.tile([128, 128], F32)
        nc.gpsimd.memset(wtile, 0.0)

        for _rep in range(reps):
            _build_body(nc, tc, sb, es, ping, accp, wtile, ones4,
                        xw_dram, xv_dram, out_dram)

    nc.compile()
    return nc


def _build_body(nc, tc, sb, es, ping, accp, wtile, ones4,
                xw_dram, xv_dram, out_dram):
    psA = ping.tile([128, 3 * SQ], F32, tag="A")
    psB = ping.tile([128, 3 * SQ], F32, tag="B")
    pst_of = lambda g: psA if g % 2 == 0 else psB
    accT = accp.tile([128, 2 * SQ], F32, tag="acc")
    accs = [accT[0:4, SQ * (j % 2): SQ * (j % 2 + 1)] for j in range(4)]

    # input DMAs: x layouts + weights, packed per-tensor (one HWDGE phase each)
    xw_sb = sb.tile([27, 2054], F32R)
    nc.sync.dma_start(xw_sb[:, 0:518], _r(xw_dram.ap()[:, 0:518]))
    nc.sync.dma_start(xw_sb[:, 518:1030], _r(xw_dram.ap()[:, 518:1030]))
    nc.sync.dma_start(xw_sb[:, 1030:2054], _r(xw_dram.ap()[:, 1030:2054]))
    xv_sb = sb.tile([82, 320], F32R)
    nc.gpsimd.dma_start(xv_sb[:], _r(xv_dram.ap()))

    # warm the PE pstate during the DMA window so the projection matmuls and
    # the first rounds run at full clock (writes are dead; overwritten later)
    for _w in range(14):
        nc.tensor.transpose(_r(psB[0:128, SQ: SQ + 128]), _r(wtile[:]), _r(wtile[:]))

    # k^T / q^T projections ([3, S] each, true t-order).  The round-0 pieces
    # stage through psB (dead until round 1 overwrites it); the rest trickle
    # through the acc banks inside the main loop.
    qT = sb.tile([3, S], F32R)
    kT = sb.tile([3, S], F32R)
    wq = xw_sb[:, 0:3]
    wk = xw_sb[:, 3:6]
    X = 6
    nc.tensor.matmul(psA[0:3, 1024:1536], lhsT=wq, rhs=xw_sb[:, X: X + 512],
                     start=True, stop=True)
    nc.vector.tensor_copy(qT[:, 0:512], psA[0:3, 1024:1536])
    nc.tensor.matmul(psB[0:3, 0:512], lhsT=wk, rhs=xw_sb[:, X: X + 512],
                     start=True, stop=True)
    nc.scalar.copy(kT[:, 0:256], psB[0:3, 0:256])
    nc.scalar.copy(kT[:, 256:512], psB[0:3, 256:512])
    nc.tensor.matmul(psB[0:3, 512:1024], lhsT=wk, rhs=xw_sb[:, X + 512: X + 1024],
                     start=True, stop=True)
    nc.vector.tensor_copy(kT[:, 512:1024], psB[0:3, 512:1024])

    vplus = sb.tile([128, 64], F32R)
    recip = sb.tile([1, S], F32R)
    bc_sb = sb.tile([4, S], F32R)
    outv = sb.tile([4, S], F32)
    accv = sb.tile([4, SQ], F32)

    def mm1(g):
        j, r = divmod(g, NR)
        pst = pst_of(g)
        for i, c in enumerate(_rc(j)[r]):
            nc.tensor.matmul(
                pst[:, SQ * i: SQ * (i + 1)],
                lhsT=kT[:, 128 * c: 128 * (c + 1)],
                rhs=qT[:, SQ * j: SQ * (j + 1)],
                start=True,
                stop=True,
            )

    def mm2(g, e_t):
        j, r = divmod(g, NR)
        for i, c in enumerate(_rc(j)[r]):
            nc.tensor.matmul(
                accs[j],
                lhsT=vplus[:, 4 * c: 4 * (c + 1)],
                rhs=e_t[:, SQ * i: SQ * (i + 1)],
                start=(r == 0 and i == 0),
                stop=(r == NR - 1 and i == len(_rc(j)[r]) - 1),
            )

    def epilogue(j):
        if j == 3:
            # tail: halved reciprocal -> GPSIMD broadcast -> multiply chain;
            # all PSUM-acc readers stay on DVE (cross-engine readers of the
            # same tile serialize anyway), so the final DMA issues early
            HQ = SQ // 4
            for h in range(4):
                cl = slice(SQ * j + HQ * h, SQ * j + HQ * (h + 1))
                with nc.allow_low_precision(reason="float32r is 4-byte"):
                    nc.vector.reciprocal(recip[:, cl], _r(accs[j][0:1, HQ * h: HQ * (h + 1)]))
                nc.gpsimd.partition_broadcast(bc_sb[0:4, cl], recip[:, cl])
            for h in range(4):
                cl = slice(SQ * j + HQ * h, SQ * j + HQ * (h + 1))
                nc.vector.tensor_mul(outv[0:4, cl], accs[j][0:4, HQ * h: HQ * (h + 1)],
                                     bc_sb[0:4, cl])
        else:
            with nc.allow_low_precision(reason="float32r is 4-byte"):
                nc.vector.reciprocal(recip[:, SQ * j: SQ * (j + 1)], _r(accs[j][0:1, :]))
            bc = bc_sb[0:4, SQ * j: SQ * (j + 1)]
            nc.gpsimd.partition_broadcast(bc, recip[:, SQ * j: SQ * (j + 1)])
            nc.vector.tensor_mul(outv[0:4, SQ * j: SQ * (j + 1)], accs[j][0:4, :], bc)
        nc.sync.dma_start(out_dram.ap()[:, SQ * j: SQ * (j + 1)],
                          outv[1:4, SQ * j: SQ * (j + 1)])

    mm1(0)

    prev_e = None
    for g in range(NG):
        j, r = divmod(g, NR)
        pst = pst_of(g)
        width = SQ * len(_rc(j)[r])
        e_t = es.tile([128, 3 * SQ], F32R)
        nc.scalar.activation(
            e_t[:, 0:width], pst[:, 0:width],
            mybir.ActivationFunctionType.Exp, scale=INV_SCALE,
        )
        # next round's qk matmuls are independent of this exp: issue them first
        # so the in-order PE stream overlaps the exp (keeps ACT back-to-back)
        if g + 1 < NG:
            mm1(g + 1)
        if g < 5:
            # remaining projection pieces ride idle PE slots one 512-col piece
            # per round, staged through the acc1 bank (dead until s-chunk 1's
            # accumulation starts at g=7) and drained by DVE copies.
            dst, off = [(kT, 1024), (kT, 1536), (qT, 512), (qT, 1024), (qT, 1536)][g]
            w = wk if dst is kT else wq
            nc.tensor.matmul(accT[0:3, 512:1024], lhsT=w,
                             rhs=xw_sb[:, X + off: X + off + 512], start=True, stop=True)
            nc.vector.tensor_copy(dst[:, off: off + 512], accT[0:3, 512:1024])
        if g == 0:
            # [1|v] quads via two 82-dim masked matmuls staged in the acc0
            # bank (dead until mm2 of round 0 starts accumulating there)
            nc.tensor.matmul(accT[0:128, 0:32], lhsT=xv_sb[:, 0:128],
                             rhs=xv_sb[:, 256:288], start=True, stop=True)
            nc.tensor.matmul(accT[0:128, 32:64], lhsT=xv_sb[:, 128:256],
                             rhs=xv_sb[:, 288:320], start=True, stop=True)
            nc.vector.tensor_copy(vplus[:], accT[0:128, 0:64])
        if g >= 1:
            mm2(g - 1, prev_e)
        if r == 0 and j >= 1:
            epilogue(j - 1)
        prev_e = e_t

    mm2(NG - 1, prev_e)
    epilogue(3)


def _host_inputs(x_h, Q, K, V):
    """Layout-only host prep: zero-masked arrangements of x plus weight
    placements that make q^T/k^T/[1|v] single PE matmuls (see docstring)."""
    t = np.arange(S)
    xw = np.zeros((27, 2054), np.float32)
    for p in range(3):
        m = 3 * t + p
        ireq = m // S
        s = m % S
        for i in range(3):
            msk = ireq == i
            for d in range(3):
                row = 9 * p + 3 * i + d
                xw[row, 6: 6 + S] = np.where(msk, x_h[s, d], 0.0)
                xw[row, p] = Q[i, d]
                xw[row, 3 + p] = K[i, d]

    pp = np.arange(128)

    def vgroup(c0):
        XV = np.zeros((82, 128), np.float32)
        RV = np.zeros((82, 32), np.float32)
        XV[0] = 1.0
        slot = 1
        for c in range(c0, c0 + 8):
            RV[0, 4 * (c - c0)] = 1.0
            ivals = sorted({(384 * c) // S, (384 * c + 383) // S})
            for r in range(3):
                sfull = 384 * c + 3 * pp + r
                ireq = sfull // S
                s = sfull % S
                for il in ivals:
                    msk = ireq == il
                    for d in range(3):
                        XV[slot] = np.where(msk, x_h[s, d], 0.0)
                        RV[slot, 4 * (c - c0) + 1 + r] = V[il, d]
                        slot += 1
        assert slot == 82, slot
        return XV, RV

    XVA, RVA = vgroup(0)
    XVB, RVB = vgroup(8)
    xv = np.concatenate([XVA, XVB, RVA, RVB], axis=1)
    return np.ascontiguousarray(xw), np.ascontiguousarray(xv)


_NC_CACHE = None


def _get_program():
    global _NC_CACHE
    if _NC_CACHE is None:
        _NC_CACHE = build_program()
    return _NC_CACHE


def kernel(x1, query, key_w, value, dropout_p=0):
    x1 = np.asarray(x1, dtype=np.float32)
    query = np.asarray(query, dtype=np.float32)
    key_w = np.asarray(key_w, dtype=np.float32)
    value = np.asarray(value, dtype=np.float32)

    in_maps = []
    for h in range(H):
        xw, xv = _host_inputs(x1[h], query[h], key_w[h], value[h])
        in_maps.append({"xw": xw, "xv": xv})

    # The axon terminal very occasionally drops a worker mid-execute
    # (NRT_EXEC_UNIT_UNRECOVERABLE); the kernel itself is deterministic, so
    # retry once with a freshly built program before giving up.
    global _NC_CACHE
    last_err = None
    for attempt in range(2):
        try:
            nc = _get_program()
            res = bass_utils.run_bass_kernel_spmd(nc, in_maps, core_ids=list(range(H)))
            return np.stack([res.results[h]["out"] for h in range(H)])
        except Exception as e:  # noqa: BLE001 - transient runtime faults only
            last_err = e
            _NC_CACHE = None
            import time as _time

            _time.sleep(5.0)
    raise last_err


# revision 21
# speedup vs baseline: 1.0012x; 1.0012x over previous
"""Trainium2 Bass kernel for nn_Model_39676907886571 (per-head attention, S=2048, d=3).

Math (per head h, fully head/data parallel, one head per NeuronCore):
  q_mat = query[h] @ x[h].T          (3, S)   -> q = q_mat viewed row-major as (S, 3)
  k_mat, v_mat likewise (the reshape is a memory-reinterpreting view, not a transpose)
  attn  = softmax(q @ k.T / sqrt(3)) (S, S)
  out   = (attn @ v).T               (3, S)

Device strategy (all on-chip, the S x S attention matrix never touches HBM):
  * The memory-reinterpreting (S,3)-view of the (3,S) projection output is a
    fixed permutation of x entries, so it is absorbed into the HOST-side input
    layout: a zero-masked 27-partition arrangement of x (k = (p', i, d) with
    the view's row-crossing i-mask baked in) turns q^T and k^T into plain
    [27,3] x [27,512] PE matmuls whose outputs are ALREADY (3, S)-transposed
    and in true sequence order.  The [1|v0|v1|v2] quads for attn@[1|v]
    likewise come from two 82-dim masked matmuls.  This removes the
    DRAM-bounce reshape, all 40 PE transposes, and the u-order un-permute of
    the earlier design; both softmax axes are true-order so the output DMA
    streams per s-chunk.
  * E^T = exp(k-chunks^T @ q^T / sqrt(3)) keeps the key axis on partitions, so
    attn @ [1|v] needs no transposes and the softmax denominator falls out of
    the ones column of the [1|v] stationary operand.
  * Matmul operands are float32r (single-pass fp32 PE mode); PSUM: 2 x 3-bank
    ping-pong qk tiles + 2 x 1-bank attn@[1|v] accumulators; the acc banks
    double as staging for the later projection pieces before their first
    accumulation starts (matmul outputs must sit at base partition 0).
  * Software pipelining: mm1 of round g+1 issues before mm2 of round g so the
    in-order PE stream overlaps the exp; ACT (exp of the 2048^2 attention
    matrix = ~31.7us floor) runs back-to-back from round 0, which starts as
    soon as the first 512-col projection pieces are copied to SBUF.  Round
    widths are 3,3,2,3,3,2 chunks (a width-1 round mid-stream starves the
    next round's mm1 behind the 2-tile PSUM ping-pong); only the program's
    last round is width 1, keeping the tail (mm2 + normalize + final DMA)
    short.  The tail normalization runs ACT (acc->SBUF copy) and DVE
    (reciprocal) in parallel, broadcasts via a PE matmul, and multiplies on
    DVE with a single PSUM operand.
"""

import numpy as np
from contextlib import ExitStack

import concourse.bass as bass
import concourse.tile as tile
from concourse import bacc, mybir
from concourse import bass_utils

F32 = mybir.dt.float32
F32R = mybir.dt.float32r

H, S, D = 8, 2048, 3
NCH = 16                # t-chunks of 128
SQ = 512                # s-chunk width (one PSUM bank)
INV_SCALE = float(1.0 / np.sqrt(3.0))

RC_EARLY = [(0, 1), (2, 3, 4), (5, 6, 7), (8, 9, 10), (11, 12), (13, 14, 15)]
RC_LAST = [(0, 1, 2), (3, 4, 5), (6, 7, 8), (9, 10, 11), (12, 13, 14), (15,)]
NR = 6
NG = 4 * NR


def _rc(j):
    return RC_LAST if j == 3 else RC_EARLY


def _r(ap):
    """Bitcast an fp32 AP to float32r (same bits)."""
    return ap.bitcast(F32R)


def build_program(reps=1):
    nc = bacc.Bacc("TRN2", num_devices=H, debug=False)
    xw_dram = nc.dram_tensor("xw", (27, 2054), F32, kind="ExternalInput")
    xv_dram = nc.dram_tensor("xv", (82, 320), F32, kind="ExternalInput")
    out_dram = nc.dram_tensor("out", (3, S), F32, kind="ExternalOutput")

    with tile.TileContext(nc) as tc, ExitStack() as ctx:
        consts = ctx.enter_context(tc.tile_pool(name="consts", bufs=1))
        sb = ctx.enter_context(tc.tile_pool(name="sb", bufs=2 if reps > 1 else 1))
        es = ctx.enter_context(tc.tile_pool(name="es", bufs=4))
        ping = ctx.enter_context(tc.tile_pool(name="ping", bufs=1, space="PSUM"))
        accp = ctx.enter_context(tc.tile_pool(name="accp", bufs=1, space="PSUM"))

        # constants (shared across reps)
        warm = consts.tile([1, 4], F32)
        nc.vector.memset(warm, 1.0)
        ones4 = consts.tile([1, 4], F32R)
        nc.vector.tensor_copy(ones4[:], warm[:])
        # prewarm the ACT exp table so the ~1.3us table load overlaps the prologue
        nc.scalar.activation(warm[0:1, 0:1], warm[0:1, 1:2],
                             mybir.ActivationFunctionType.Exp)
        # memset on the otherwise-idle GPSIMD so the dead warm-up transposes
        # are ready almost at t=0 (the PE pstate ramp clock starts then)
        wtile = consts.tile([128, 128], F32)
        nc.gpsimd.memset(wtile, 0.0)

        for _rep in range(reps):
            _build_body(nc, tc, sb, es, ping, accp, wtile, ones4,
                        xw_dram, xv_dram, out_dram)

    nc.compile()
    return nc


def _build_body(nc, tc, sb, es, ping, accp, wtile, ones4,
                xw_dram, xv_dram, out_dram):
    psA = ping.tile([128, 3 * SQ], F32, tag="A")
    psB = ping.tile([128, 3 * SQ], F32, tag="B")
    pst_of = lambda g: psA if g % 2 == 0 else psB
    accT = accp.tile([128, 2 * SQ], F32, tag="acc")
    accs = [accT[0:4, SQ * (j % 2): SQ * (j % 2 + 1)] for j in range(4)]

    # input DMAs: x layouts + weights, packed per-tensor (one HWDGE phase each)
    xw_sb = sb.tile([27, 2054], F32R)
    nc.sync.dma_start(xw_sb[:, 0:518], _r(xw_dram.ap()[:, 0:518]))
    nc.sync.dma_start(xw_sb[:, 518:1030], _r(xw_dram.ap()[:, 518:1030]))
    nc.sync.dma_start(xw_sb[:, 1030:2054], _r(xw_dram.ap()[:, 1030:2054]))
    xv_sb = sb.tile([82, 320], F32R)
    nc.gpsimd.dma_start(xv_sb[:], _r(xv_dram.ap()))

    # warm the PE pstate during the DMA window so the projection matmuls and
    # the first rounds run at full clock (writes are dead; overwritten later)
    for _w in range(14):
        nc.tensor.transpose(_r(psB[0:128, SQ: SQ + 128]), _r(wtile[:]), _r(wtile[:]))

    # k^T / q^T projections ([3, S] each, true t-order).  The round-0 pieces
    # stage through psB (dead until round 1 overwrites it); the rest trickle
    # through the acc banks inside the main loop.
    qT = sb.tile([3, S], F32R)
    kT = sb.tile([3, S], F32R)
    wq = xw_sb[:, 0:3]
    wk = xw_sb[:, 3:6]
    X = 6
    nc.tensor.matmul(psA[0:3, 1024:1536], lhsT=wq, rhs=xw_sb[:, X: X + 512],
                     start=True, stop=True)
    nc.vector.tensor_copy(qT[:, 0:512], psA[0:3, 1024:1536])
    nc.tensor.matmul(psB[0:3, 0:512], lhsT=wk, rhs=xw_sb[:, X: X + 512],
                     start=True, stop=True)
    nc.scalar.copy(kT[:, 0:256], psB[0:3, 0:256])
    nc.scalar.copy(kT[:, 256:512], psB[0:3, 256:512])
    nc.tensor.matmul(psB[0:3, 512:1024], lhsT=wk, rhs=xw_sb[:, X + 512: X + 1024],
                     start=True, stop=True)
    nc.vector.tensor_copy(kT[:, 512:1024], psB[0:3, 512:1024])

    vplus = sb.tile([128, 64], F32R)
    recip = sb.tile([1, S], F32R)
    bc_sb = sb.tile([4, S], F32R)
    outv = sb.tile([4, S], F32)
    accv = sb.tile([4, SQ], F32)

    def mm1(g):
        j, r = divmod(g, NR)
        pst = pst_of(g)
        for i, c in enumerate(_rc(j)[r]):
            nc.tensor.matmul(
                pst[:, SQ * i: SQ * (i + 1)],
                lhsT=kT[:, 128 * c: 128 * (c + 1)],
                rhs=qT[:, SQ * j: SQ * (j + 1)],
                start=True,
                stop=True,
            )

    def mm2(g, e_t):
        j, r = divmod(g, NR)
        for i, c in enumerate(_rc(j)[r]):
            nc.tensor.matmul(
                accs[j],
                lhsT=vplus[:, 4 * c: 4 * (c + 1)],
                rhs=e_t[:, SQ * i: SQ * (i + 1)],
                start=(r == 0 and i == 0),
                stop=(r == NR - 1 and i == len(_rc(j)[r]) - 1),
            )

    def epilogue(j):
        if j == 3:
            # tail: halved reciprocal -> GPSIMD broadcast -> multiply chain;
            # all PSUM-acc readers stay on DVE (cross-engine readers of the
            # same tile serialize anyway), so the final DMA issues early
            HQ = SQ // 4
            for h in range(4):
                cl = slice(SQ * j + HQ * h, SQ * j + HQ * (h + 1))
                with nc.allow_low_precision(reason="float32r is 4-byte"):
                    nc.vector.reciprocal(recip[:, cl], _r(accs[j][0:1, HQ * h: HQ * (h + 1)]))
                nc.gpsimd.partition_broadcast(bc_sb[0:4, cl], recip[:, cl])
            for h in range(4):
                cl = slice(SQ * j + HQ * h, SQ * j + HQ * (h + 1))
                nc.vector.tensor_mul(outv[0:4, cl], accs[j][0:4, HQ * h: HQ * (h + 1)],
                                     bc_sb[0:4, cl])
        else:
            with nc.allow_low_precision(reason="float32r is 4-byte"):
                nc.vector.reciprocal(recip[:, SQ * j: SQ * (j + 1)], _r(accs[j][0:1, :]))
            bc = bc_sb[0:4, SQ * j: SQ * (j + 1)]
            nc.gpsimd.partition_broadcast(bc, recip[:, SQ * j: SQ * (j + 1)])
            nc.vector.tensor_mul(outv[0:4, SQ * j: SQ * (j + 1)], accs[j][0:4, :], bc)
        nc.sync.dma_start(out_dram.ap()[:, SQ * j: SQ * (j + 1)],
                          outv[1:4, SQ * j: SQ * (j + 1)])

    mm1(0)

    prev_e = None
    for g in range(NG):
        j, r = divmod(g, NR)
        pst = pst_of(g)
        width = SQ * len(_rc(j)[r])
        e_t = es.tile([128, 3 * SQ], F32R)
        nc.scalar.activation(
            e_t[:, 0:width], pst[:, 0:width],
            mybir.ActivationFunctionType.Exp, scale=INV_SCALE,
        )
        # next round's qk matmuls are independent of this exp: issue them first
        # so the in-order PE stream overlaps the exp (keeps ACT back-to-back)
        if g + 1 < NG:
            mm1(g + 1)
        if g < 5:
            # remaining projection pieces ride idle PE slots one 512-col piece
            # per round, staged through the acc1 bank (dead until s-chunk 1's
            # accumulation starts at g=7) and drained by DVE copies.
            dst, off = [(kT, 1024), (kT, 1536), (qT, 512), (qT, 1024), (qT, 1536)][g]
            w = wk if dst is kT else wq
            nc.tensor.matmul(accT[0:3, 512:1024], lhsT=w,
                             rhs=xw_sb[:, X + off: X + off + 512], start=True, stop=True)
            nc.vector.tensor_copy(dst[:, off: off + 512], accT[0:3, 512:1024])
        if g == 0:
            # [1|v] quads via two 82-dim masked matmuls staged in the acc0
            # bank (dead until mm2 of round 0 starts accumulating there)
            nc.tensor.matmul(accT[0:128, 0:32], lhsT=xv_sb[:, 0:128],
                             rhs=xv_sb[:, 256:288], start=True, stop=True)
            nc.tensor.matmul(accT[0:128, 32:64], lhsT=xv_sb[:, 128:256],
                             rhs=xv_sb[:, 288:320], start=True, stop=True)
            nc.vector.tensor_copy(vplus[:], accT[0:128, 0:64])
        if g >= 1:
            mm2(g - 1, prev_e)
        if r == 0 and j >= 1:
            epilogue(j - 1)
        prev_e = e_t

    mm2(NG - 1, prev_e)
    epilogue(3)


def _host_inputs(x_h, Q, K, V):
    """Layout-only host prep: zero-masked arrangements of x plus weight
    placements that make q^T/k^T/[1|v] single PE matmuls (see docstring)."""
    t = np.arange(S)
    xw = np.zeros((27, 2054), np.float32)
    for p in range(3):
        m = 3 * t + p
        ireq = m // S
        s = m % S
        for i in range(3):
            msk = ireq == i
            for d in range(3):
                row = 9 * p + 3 * i + d
                xw[row, 6: 6 + S] = np.where(msk, x_h[s, d], 0.0)
                xw[row, p] = Q[i, d]
                xw[row, 3 + p] = K[i, d]

    pp = np.arange(128)

    def vgroup(c0):
        XV = np.zeros((82, 128), np.float32)
        RV = np.zeros((82, 32), np.float32)
        XV[0] = 1.0
        slot = 1
        for c in range(c0, c0 + 8):
            RV[0, 4 * (c - c0)] = 1.0
            ivals = sorted({(384 * c) // S, (384 * c + 383) // S})
            for r in range(3):
                sfull = 384 * c + 3 * pp + r
                ireq = sfull // S
                s = sfull % S
                for il in ivals:
                    msk = ireq == il
                    for d in range(3):
                        XV[slot] = np.where(msk, x_h[s, d], 0.0)
                        RV[slot, 4 * (c - c0) + 1 + r] = V[il, d]
                        slot += 1
        assert slot == 82, slot
        return XV, RV

    XVA, RVA = vgroup(0)
    XVB, RVB = vgroup(8)
    xv = np.concatenate([XVA, XVB, RVA, RVB], axis=1)
    return np.ascontiguousarray(xw), np.ascontiguousarray(xv)


_NC_CACHE = None


def _get_program():
    global _NC_CACHE
    if _NC_CACHE is None:
        _NC_CACHE = build_program()
    return _NC_CACHE


def kernel(x1, query, key_w, value, dropout_p=0):
    x1 = np.asarray(x1, dtype=np.float32)
    query = np.asarray(query, dtype=np.float32)
    key_w = np.asarray(key_w, dtype=np.float32)
    value = np.asarray(value, dtype=np.float32)

    in_maps = []
    for h in range(H):
        xw, xv = _host_inputs(x1[h], query[h], key_w[h], value[h])
        in_maps.append({"xw": xw, "xv": xv})

    # The axon terminal very occasionally drops a worker mid-execute
    # (NRT_EXEC_UNIT_UNRECOVERABLE); the kernel itself is deterministic, so
    # retry once with a freshly built program before giving up.
    global _NC_CACHE
    last_err = None
    for attempt in range(2):
        try:
            nc = _get_program()
            res = bass_utils.run_bass_kernel_spmd(nc, in_maps, core_ids=list(range(H)))
            return np.stack([res.results[h]["out"] for h in range(H)])
        except Exception as e:  # noqa: BLE001 - transient runtime faults only
            last_err = e
            _NC_CACHE = None
            import time as _time

            _time.sleep(5.0)
    raise last_err
